# revision 1
# baseline (speedup 1.0000x reference)
"""Trainium2 Bass kernel for nn_ExteriorDerivative (d of a 2-form via central FD).

Math: the reference's central finite difference collapses analytically:
  (x +/- eps e_d) @ W1 = z +/- eps*W1[d]  with z = x @ W1, and
  sin(z+a) - sin(z-a) = 2 cos(z) sin(a), so
  fd[d] = cos(z) @ (diag(sin(eps*W1[d])/eps) @ W2)
and the whole gather/sign/scatter pipeline folds into one (32, 35) matrix G:
  out = cos(x @ W1) @ G.

On-device cos: ACT Sin is only accurate on [-pi, pi] and |z| < 2*pi here, so
  cos(z) = 1 - 2*sin^2(z/2),   |z/2| < pi  (no range reduction needed)
The z/2 is Sin's free scale; the square is one DVE multiply; the affine
1 - 2*(.) folds into mm2 (-2*G as the weights) plus a per-output-row
constant sum_j G[j,o] added for free during the PSUM->SBUF output copy
(tensor_scalar add / Identity-activation bias).

Device pipeline per core (pure batch-parallel across 8 cores; host packs x
into a block-diagonal-ready layout, 3 batch subgroups stacked on partitions,
batch along the free dim):
  mm1: z^T = blockdiag(W1, x3)^T @ xt               [96, N] PSUM (float32r)
  s = Sin(0.5 * z)                                  ACT, PSUM->SBUF
  q = s * s                                         DVE, SBUF->SBUF
  mm2: out^T = (-2*Gblk)^T @ q                      [105, N] PSUM (f32r/fp16)
  copy+add g1 PSUM->SBUF (DVE/ACT), DMA out         ot [105, COLS]
Host unshuffles ot back to (B, 35).
"""
import numpy as np
from itertools import combinations

DIM = 7
EPS = 1e-4
NCORES = 8
B = 262144
B_CORE = B // NCORES          # 32768
SUB = 3                       # block-diagonal subgroups
K_IN = 3 * DIM                # 21 data partitions
M1 = 3 * 32                   # 96 z dims
TILE_N = 512                  # matmul moving-operand max (fp32 psum bank)
GROUP_N = 1024                # cols per psum group (2 matmuls per group)
COLS = 11264                  # columns per core (22 * 512)
B_CORE_PAD = SUB * COLS       # 33792
CHUNK_COLS = 2048             # cols per DMA chunk
COPY_MOD = 3                  # every COPY_MOD-th output copy goes to ACT
ZBUFS = 2
OBUFS = 2
WBUFS = 4
REPS = 1                      # in-kernel repeats (timing only)
import os as _os
SCHED_V2 = 0
SQ_GPS = set(int(v) for v in _os.environ.get("K_SQGPS", "5").split(",") if v)
_ca = _os.environ.get("K_COPYACT", "")
COPY_ACT = (set(int(v) for v in _ca.split(",") if v) if _ca
            else None)        # None -> COPY_MOD rule

USE_F32R = True               # float32r matmuls (4x faster than fp32 on PE)

# ---- static exterior-derivative index maps (mirrors reference.py) ----
_IDX3 = list(combinations(range(DIM), 3))
_POS2 = {t: i for i, t in enumerate(combinations(range(DIM), 2))}
_D2 = []
for _out, (i, j, k) in enumerate(_IDX3):
    for _p, (a, b, c) in enumerate([(i, j, k), (j, i, k), (k, i, j)]):
        bc = tuple(sorted((b, c)))
        s = (-1) ** _p * (1 if (b, c) == bc else -1)
        _D2.append((_out, _POS2[bc], a, s))


def _build_G(W1: np.ndarray, W2: np.ndarray) -> np.ndarray:
    """G[j, o] = sum_t SIGNS[t] * sin(EPS*W1[DCOORD[t], j])/EPS * W2[j, IN_POS[t]]  (fp64)."""
    W1d = W1.astype(np.float64)
    W2d = W2.astype(np.float64)
    G = np.zeros((32, 35), dtype=np.float64)
    for out_pos, in_pos, dcoord, sign in _D2:
        G[:, out_pos] += sign * (np.sin(EPS * W1d[dcoord, :]) / EPS) * W2d[:, in_pos]
    return G


_PROG = None


def _get_prog(reps=None):
    global _PROG
    if reps is None:
        reps = REPS
    if _PROG is not None and reps == REPS:
        return _PROG
    import concourse.bacc as bacc
    import concourse.bass as bass
    import concourse.tile as tile
    import concourse.mybir as mybir

    F32 = mybir.dt.float32
    F16 = mybir.dt.float16
    FMM = mybir.dt.float32r if USE_F32R else F32
    Sin = mybir.ActivationFunctionType.Sin
    Ident = mybir.ActivationFunctionType.Identity
    Alu = mybir.AluOpType

    nc = bacc.Bacc("TRN2", target_bir_lowering=False, debug=False, num_devices=NCORES)
    xt = nc.dram_tensor("xt", [K_IN, COLS], FMM, kind="ExternalInput")
    w1b = nc.dram_tensor("w1b", [K_IN, M1], FMM, kind="ExternalInput")
    gb = nc.dram_tensor("gb", [M1, 105], F16, kind="ExternalInput")
    ot = nc.dram_tensor("ot", [105, COLS], F16, kind="ExternalOutput")

    with tile.TileContext(nc) as tc:
        with (
            tc.tile_pool(name="singles", bufs=1) as singles,
            tc.tile_pool(name="xin", bufs=4) as xpool,
            tc.tile_pool(name="och", bufs=4) as opool,
            tc.tile_pool(name="work", bufs=WBUFS) as wpool,
            tc.tile_pool(name="zps", bufs=ZBUFS, space=bass.MemorySpace.PSUM) as zpsum,
            tc.tile_pool(name="ops", bufs=OBUFS, space=bass.MemorySpace.PSUM) as opsum,
        ):
            w1s = singles.tile([K_IN, M1], FMM)
            nc.gpsimd.dma_start(w1s[:], w1b[:])
            gs = singles.tile([M1, 105], F16)
            nc.gpsimd.dma_start(gs[:], gb[:])
            g1b = nc.dram_tensor("g1b", [105, 1], F32, kind="ExternalInput")
            g1s = singles.tile([105, 1], F32)
            nc.gpsimd.dma_start(g1s[:], g1b[:])
            zbias = singles.tile([M1, 1], F32)
            nc.vector.memset(zbias[:], 0.0)

            # (chunk_cols, group_cols) schedule: small groups at the start
            # (fast pipeline fill) and end (short tail chain)
            if SCHED_V2 == 4:
                sched = [(512, 512)] + [(2048, 1024)] * 5 + [(512, 512)]
            elif SCHED_V2 == 5:
                sched = ([(512, 512), (1024, 1024)] + [(2048, 1024)] * 4
                         + [(1024, 1024), (512, 512)])
            elif SCHED_V2 == 1:
                sched = [(1024, 512)] + [(2048, 1024)] * 4 + [(1024, 1024), (1024, 512)]
            elif SCHED_V2 == 2:
                sched = [(2048, 1024)] * 5 + [(1024, 512)]
            elif SCHED_V2 == 3:
                sched = [(1024, 512)] + [(2048, 1024)] * 5
            else:
                sched = []
                rem = COLS
                while rem > 0:
                    c = min(CHUNK_COLS, rem)
                    sched.append((c, min(GROUP_N, c)))
                    rem -= c
            assert sum(c for c, _ in sched) == COLS
            gidx = 0
            for _rep in range(reps):
              c0 = 0
              for (cn, gcols) in sched:
                  # loads on HWDGE/sync (SP does nothing else -> no
                  # head-of-line blocking); stores go out via SWDGE/gpsimd
                  xin = xpool.tile([K_IN, cn], FMM, tag="xin")
                  nc.sync.dma_start(xin[:, :cn], xt[:, c0:c0 + cn])
                  och = opool.tile([105, cn], F16, tag="och")
                  for g0 in range(0, cn, gcols):
                      gn = min(gcols, cn - g0)
                      zp = zpsum.tile([M1, GROUP_N], F32, tag="zp")
                      for s0 in range(0, gn, TILE_N):
                          nc.tensor.matmul(zp[:, s0:s0 + TILE_N], w1s[:],
                                           xin[:, g0 + s0:g0 + s0 + TILE_N])
                      ss = wpool.tile([M1, GROUP_N], F16, tag="ss")
                      nc.scalar.activation(ss[:, :gn], zp[:, :gn], Sin,
                                           bias=zbias[:], scale=0.5)
                      qq = wpool.tile([M1, GROUP_N], F16, tag="qq")
                      sq_eng = nc.gpsimd if gidx in SQ_GPS else nc.vector
                      sq_eng.tensor_tensor(qq[:, :gn], ss[:, :gn], ss[:, :gn],
                                           Alu.mult)
                      op = opsum.tile([105, GROUP_N], F32, tag="op")
                      for s0 in range(0, gn, TILE_N):
                          nc.tensor.matmul(op[:, s0:s0 + TILE_N], gs[:],
                                           qq[:, s0:s0 + TILE_N])
                      # PSUM->SBUF output copy fused with the "+sum_j G" term
                      # (cos = 1 - 2 sin^2 reconstruction); split DVE/ACT
                      on_act = (gidx in COPY_ACT) if COPY_ACT is not None else (
                          COPY_MOD and gidx % COPY_MOD == COPY_MOD - 1)
                      if on_act:
                          nc.scalar.activation(och[:, g0:g0 + gn], op[:, :gn],
                                               Ident, bias=g1s[:], scale=1.0)
                      else:
                          nc.vector.tensor_scalar(och[:, g0:g0 + gn], op[:, :gn],
                                                  g1s[:], None, Alu.add)
                      gidx += 1
                  nc.gpsimd.dma_start(ot[:, c0:c0 + cn], och[:, :cn])
                  c0 += cn

    nc.compile()
    if reps == REPS:
        _PROG = nc
    return nc


def _pack_inputs(x: np.ndarray, W1: np.ndarray, W2: np.ndarray):
    assert x.shape == (B, DIM), x.shape
    assert W1.shape == (DIM, 32), W1.shape
    assert W2.shape == (32, 21), W2.shape
    G = _build_G(W1, W2)                      # fp64 (32, 35)
    g1 = G.sum(axis=0)                        # (35,)
    W1blk = np.zeros((K_IN, M1), dtype=np.float32)
    Gblk = np.zeros((M1, 105), dtype=np.float16)
    g1blk = np.zeros((105, 1), dtype=np.float32)
    for i in range(SUB):
        W1blk[7 * i:7 * i + 7, 32 * i:32 * i + 32] = W1
        Gblk[32 * i:32 * i + 32, 35 * i:35 * i + 35] = -2.0 * G
        g1blk[35 * i:35 * i + 35, 0] = g1

    xpad = np.zeros((NCORES, B_CORE_PAD, DIM), dtype=np.float32)
    xpad[:, :B_CORE, :] = np.ascontiguousarray(x, dtype=np.float32).reshape(NCORES, B_CORE, DIM)
    # xt[m][7*i + f, c] = xpad[m, i*COLS + c, f] ; row 21 = ones
    xt = np.ascontiguousarray(
        xpad.reshape(NCORES, SUB, COLS, DIM).transpose(0, 1, 3, 2).reshape(NCORES, 21, COLS))
    in_maps = [{"xt": xt[m], "w1b": W1blk, "gb": Gblk, "g1b": g1blk} for m in range(NCORES)]
    return in_maps


def _unpack_outputs(results) -> np.ndarray:
    # ot[m][35*i + o, c] -> out[m*B_CORE + i*COLS + c, o]
    ot = np.stack([r["ot"] for r in results])  # (8, 105, COLS)
    out = ot.reshape(NCORES, SUB, 35, COLS).transpose(0, 1, 3, 2)  # (8, 3, COLS, 35)
    out = out.reshape(NCORES, B_CORE_PAD, 35)[:, :B_CORE, :]
    return np.ascontiguousarray(out.reshape(B, 35), dtype=np.float32)


def run(x, W1, W2, **spmd_kwargs):
    """Run the kernel; returns (output, BassKernelResults)."""
    from concourse.bass_utils import run_bass_kernel_spmd
    nc = _get_prog()
    in_maps = _pack_inputs(np.asarray(x, dtype=np.float32),
                           np.asarray(W1, dtype=np.float32),
                           np.asarray(W2, dtype=np.float32))
    res = run_bass_kernel_spmd(nc, in_maps, core_ids=list(range(NCORES)), **spmd_kwargs)
    return _unpack_outputs(res.results), res


def kernel(x, W1, W2):
    out, _ = run(x, W1, W2)
    return out

